# revision 89
# baseline (speedup 1.0000x reference)
"""Trainium2 Bass kernel for GQA causal attention (B=2, L=2048, D=2048, H=16, KVH=4).

Sharding: 8 cores = 2-way data-parallel (batch) x 4-way tensor-parallel (heads).
Each core handles one batch element, 4 query heads, and the single KV head those
queries share. Wo is row-sharded; the host sums the 4 partial outputs per batch.

v2: all projection + Wo matmuls run as fp8e4 DoubleRow instructions (0.5
cycles/row) using a hi+lo error-split: operand A ~ Ah + Al (both e4m3), and
A@B ~ Ah@Bh + Al@Bh + Ah@Bl. Contraction chunks are packed in PAIRS into the
DoubleRow slice axis ([128, 2, free] operands), so each 256-deep pair takes 3
instructions at 0.5*free cycles each = 0.75x the bf16 cycle count, at ~bf16
accuracy (residual term Al@Bl ~ 0.1%). x/Wq/Wk/Wv/Wo ship from the host as
fp8 hi+lo pairs (same DMA bytes as bf16). Scores and attn@v stay bf16
(exp-produced weights can't be split cheaply).

Device-side layout trick: everything is computed transposed.  The host passes
x^T; Q/K are produced as qT/kT [head_dim, L] directly from the projection
matmuls; scores are computed transposed (sT[k, q]), so the exp'd attention
weights land as attnT [k, q] which is exactly the operand orientation the
attn@v matmul needs; attn@v yields attn_outT [d, q], exactly the lhsT the Wo
matmul needs. Zero on-device transposes.

RoPE: the host permutes Wq/Wk columns within each head so interleaved pairs
(even, odd) land in partitions [0:64) and [64:128) of qT/kT; rotation becomes
contiguous half-tile DVE ops. The permutation is orthogonal-invariant for the
q.k dot products and does not touch V or Wo.

Softmax: no max subtraction (scores are O(+-4) here). Causal structure is
block-skipped above the diagonal; diagonal k tiles compute only the causally
live column range and a gpsimd affine_select zeroes the residual intra-tile
triangle. Row sums are accumulated across k tiles on the DVE (bf16 adds) and
reduced with a gpsimd partition_all_reduce ([128,512] colsum broadcast to all
partitions, fp32 internal); the DVE reciprocal is applied to the attention
output, and the normalized output is written as fp8 hi (scalar engine copy) +
lo (DVE sub) pairs feeding the DoubleRow Wo matmuls.

Scheduling: a single instruction-emission pipeline keeps the (in-order) PE
dense. Eager phase = per chunk-pair, K(4 blocks) + V(lt 0-3) 3-term batches
tracking the streaming xh/xl pair arrivals, then Q block-0 heads 0-1.
Everything else (remaining V/Q projections, every block's Wo matmuls) is
"fill" work in a FIFO of generators drained a few micro-ops per attention
tile; staggered force-drains (heads 2/3 of the prior block) keep emission
ahead of data needs without boundary spikes. Per-head finalization
(all_reduce -> reciprocal -> normalize + fp8 hi/lo split) is staged across
mk 0/1/2/3 of the next head's tile loop so neither the PE nor the scalar
exp queue waits on it. Wo PSUM->SBUF bounces alternate DVE/scalar; output
rows batch four tiles into one 512KB DMA. Block 3 runs heads in order
[2,3,0,1] and its Wo generator is software-pipelined pair-1-first across
the fill/scores/aout psum pools, so the final head's ~2.5us finalize
latency and the kernel tail overlap ready matmuls.

Cost-model timeline (CoreSim): 205966ns (bf16 v1) -> 175468ns. PE busy
~161us (~92%), Activation ~150, DVE ~137; rel err 0.0028 (vs 0.0035 bf16).
Remaining idle: ~2.9us DMA lead-in, ~1.5us last-head finalize chain,
~4.5us output-copy/DMA/barrier tail, ~5us scattered sub-0.5us stalls.
"""

import sys

for _p in ("/opt/trn_rl_repo",):
    if _p not in sys.path:
        sys.path.insert(0, _p)

import numpy as np
import ml_dtypes

import concourse.bass as bass
import concourse.bacc as bacc
import concourse.mybir as mybir
from concourse import bass_isa
from concourse.tile import TileContext
from concourse import bass_utils

B, L, D = 2, 2048, 2048
H, KVH = 16, 4
HD = D // H            # 128
N_REP = H // KVH       # 4
TP = 4                 # tensor-parallel width (heads)
HQ = H // TP           # 4 query heads per core
SCALE = 1.0 / float(np.sqrt(HD))
# Host-side weight scaling: W ~ N(0, 0.02^2) sits in e4m3's subnormal range
# (min normal 2^-6), which destroys the lo residual of the hi+lo fp8 split.
# Scale the weights into the normal range; compensate in the exp scale
# (scores carry WS_QK^2) and divide the output partials by WS_V*WS_O on
# host. WS_V is smaller: early (short) softmax rows make ao ~ v, and
# |v|*WS_V must stay below e4m3's +-240.
WS_QK = 128.0
WS_V = 32.0
WS_O = 128.0

F32 = mybir.dt.float32
BF16 = mybir.dt.bfloat16
F8 = mybir.dt.float8e4
BF = ml_dtypes.bfloat16
E4 = ml_dtypes.float8_e4m3
DR = mybir.MatmulPerfMode.DoubleRow

NKD = D // 128         # 16 contraction chunks for projections
NPD = NKD // 2         # 8 chunk pairs
NLT = L // 128         # 16 sequence tiles of 128
NQT = L // 512         # 4 sequence tiles of 512


def qsl_of(nq):
    return slice(nq * 512, (nq + 1) * 512)


def build_nc():
    nc = bacc.Bacc(
        "TRN2",
        target_bir_lowering=False,
        debug=False,
        enable_asserts=False,
        num_devices=8,
    )

    # fp8 hi/lo inputs. x ships in natural [D, L] layout (per-pair-slice
    # transfers for arrival tracking); the weights ship PRE-PACKED into the
    # DoubleRow pair layout [128, pairs*2*width] so each is one DMA transfer
    # (64 small SWDGE transfers at the 500ns floor starved the eager phase).
    xh_d = nc.dram_tensor("xh", [D, L], F8, kind="ExternalInput")
    xl_d = nc.dram_tensor("xl", [D, L], F8, kind="ExternalInput")
    wqh_d = nc.dram_tensor("wqh", [128, NPD * 2 * HQ * HD], F8, kind="ExternalInput")
    wql_d = nc.dram_tensor("wql", [128, NPD * 2 * HQ * HD], F8, kind="ExternalInput")
    wkh_d = nc.dram_tensor("wkh", [128, NPD * 2 * HD], F8, kind="ExternalInput")
    wkl_d = nc.dram_tensor("wkl", [128, NPD * 2 * HD], F8, kind="ExternalInput")
    wvh_d = nc.dram_tensor("wvh", [128, NPD * 2 * HD], F8, kind="ExternalInput")
    wvl_d = nc.dram_tensor("wvl", [128, NPD * 2 * HD], F8, kind="ExternalInput")
    woh_d = nc.dram_tensor("woh", [128, 2 * 2 * D], F8, kind="ExternalInput")
    wol_d = nc.dram_tensor("wol", [128, 2 * 2 * D], F8, kind="ExternalInput")
    cosT = nc.dram_tensor("cosT", [HD // 2, L], BF16, kind="ExternalInput")
    sinT = nc.dram_tensor("sinT", [HD // 2, L], BF16, kind="ExternalInput")
    out = nc.dram_tensor("out", [L, D], BF16, kind="ExternalOutput")

    with TileContext(nc) as tc:
        with (
            tc.tile_pool(name="consts", bufs=1) as consts,
            tc.tile_pool(name="xw", bufs=1) as xw,
            tc.tile_pool(name="qkv", bufs=1) as qkv,
            tc.tile_pool(name="attn_sb", bufs=6) as attn_sb,
            tc.tile_pool(name="rope_t", bufs=2) as rope_t,
            tc.tile_pool(name="fin_sb", bufs=2) as fin_sb,
            tc.tile_pool(name="out_sb", bufs=4) as out_sb,
        ):
            # ---- constants ----
            cos_t = consts.tile([HD // 2, L], BF16, tag="cos")
            sin_t = consts.tile([HD // 2, L], BF16, tag="sin")

            # ---- weight + activation loads.
            # SWDGE (gpsimd): packed wk, wv (gate eager K/V), cos/sin, then
            # packed wo. HWDGE on THREE queues (sync, scalar, vector): x
            # pair-slices in pair order (pair 0 hi split into 512-col pieces
            # for an early start), then packed wq halves.
            wkh_a = xw.tile([128, NPD, 2, HD], F8, tag="wkh", name="wkh_a")
            wkl_a = xw.tile([128, NPD, 2, HD], F8, tag="wkl", name="wkl_a")
            wvh_a = xw.tile([128, NPD, 2, HD], F8, tag="wvh", name="wvh_a")
            wvl_a = xw.tile([128, NPD, 2, HD], F8, tag="wvl", name="wvl_a")
            # wkh gates the very first matmul: HWDGE (sync) beats the SWDGE
            # queue's per-transfer setup latency by ~1us; the first half
            # (pairs 0-3) ships alone so matmuls start on a quarter transfer.
            _wkhf = NPD // 2 * 2 * HD
            nc.sync.dma_start(
                wkh_a[:, 0:NPD // 2].rearrange("p a b c -> p (a b c)"),
                wkh_d[:, 0:_wkhf])
            nc.scalar.dma_start(
                wkh_a[:, NPD // 2:].rearrange("p a b c -> p (a b c)"),
                wkh_d[:, _wkhf:])
            for t, d in ((wkl_a, wkl_d),
                         (wvh_a, wvh_d), (wvl_a, wvl_d)):
                nc.gpsimd.dma_start(
                    t[:].rearrange("p a b c -> p (a b c)"), d[:])
            nc.gpsimd.dma_start(cos_t[:], cosT[:])
            nc.gpsimd.dma_start(sin_t[:], sinT[:])
            # packed wq rides the SWDGE queue (only 2 HWDGE queues exist and
            # x saturates both); halves so hi finishes before lo starts.
            wqh_a = xw.tile([128, NPD, 2, HQ * HD], F8, tag="wqh", name="wqh_a")
            wql_a = xw.tile([128, NPD, 2, HQ * HD], F8, tag="wql", name="wql_a")
            HW2 = 2 * HQ * HD
            for t, d in ((wqh_a, wqh_d), (wql_a, wql_d)):
                half = NPD // 2 * HW2
                nc.gpsimd.dma_start(
                    t[:, 0:NPD // 2].rearrange("p a b c -> p (a b c)"),
                    d[:, 0:half])
                nc.gpsimd.dma_start(
                    t[:, NPD // 2:].rearrange("p a b c -> p (a b c)"),
                    d[:, half:])
            wqh_t = [wqh_a[:, p] for p in range(NPD)]
            wql_t = [wql_a[:, p] for p in range(NPD)]
            woh_a = xw.tile([128, 2, 2, D], F8, tag="woh", name="woh_a")
            wol_a = xw.tile([128, 2, 2, D], F8, tag="wol", name="wol_a")
            for t, d in ((woh_a, woh_d), (wol_a, wol_d)):
                nc.gpsimd.dma_start(
                    t[:].rearrange("p a b c -> p (a b c)"), d[:])
            wkh_t = [wkh_a[:, p] for p in range(NPD)]
            wkl_t = [wkl_a[:, p] for p in range(NPD)]
            wvh_t = [wvh_a[:, p] for p in range(NPD)]
            wvl_t = [wvl_a[:, p] for p in range(NPD)]
            woh_t = [woh_a[:, p] for p in range(2)]
            wol_t = [wol_a[:, p] for p in range(2)]

            # x pair tiles, round-robin across the two HWDGE queues in
            # need order. pair 0 hi split into four [128, 2, 512] pieces.
            eng2 = [nc.sync, nc.scalar]
            _eq = [0]

            def hw_dma(dst, src):
                eng2[_eq[0] % 2].dma_start(dst, src)
                _eq[0] += 1

            xh0_p = []
            for pc in range(2):
                t = xw.tile([128, 2, 1024], F8, tag=f"xh0p{pc}", name=f"xh0p{pc}")
                xh0_p.append(t)
            for pc in range(2):
                c = slice(pc * 1024, (pc + 1) * 1024)
                hw_dma(xh0_p[pc][:, 0, :], xh_d[0:128, c])
                hw_dma(xh0_p[pc][:, 1, :], xh_d[128:256, c])
            xh_t = [None]
            xl_t = []
            t = xw.tile([128, 2, L], F8, tag="xl0", name="xl0")
            hw_dma(t[:, 0, :], xl_d[0:128, :])
            hw_dma(t[:, 1, :], xl_d[128:256, :])
            xl_t.append(t)
            for p in range(1, NPD):
                th = xw.tile([128, 2, L], F8, tag=f"xh{p}", name=f"xh{p}")
                tl = xw.tile([128, 2, L], F8, tag=f"xl{p}", name=f"xl{p}")
                for i in range(2):
                    r = slice((2 * p + i) * 128, (2 * p + i + 1) * 128)
                    hw_dma(th[:, i, :], xh_d[r, :])
                for i in range(2):
                    r = slice((2 * p + i) * 128, (2 * p + i + 1) * 128)
                    hw_dma(tl[:, i, :], xl_d[r, :])
                xh_t.append(th)
                xl_t.append(tl)



            # persistent activations
            kT_t = qkv.tile([128, L], BF16, tag="kT", name="kT")
            qT_t = [qkv.tile([128, L], BF16, tag=f"qT{h}", name=f"qT{h}") for h in range(HQ)]
            v_t = [qkv.tile([128, HD], BF16, tag=f"v{i}", name=f"v{i}") for i in range(NLT)]
            # attn-out as fp8 hi/lo pair tiles: P holds heads (2P, 2P+1)
            aoh_t = [qkv.tile([128, 2, L], F8, tag=f"aoh{p}", name=f"aoh{p}") for p in range(2)]
            aol_t = [qkv.tile([128, 2, L], F8, tag=f"aol{p}", name=f"aol{p}") for p in range(2)]

            def rope_store(ps, dst, sl, dve_bounce=False):
                # ps: [128, w] psum fp32 pre-rope (perm'd pairs: even rows 0:64,
                # odd rows 64:128). Bounce PSUM->SBUF once on the scalar engine
                # so the six rope DVE ops all run at SBUF rates.
                cs = cos_t[:, sl]
                sn = sin_t[:, sl]
                w = ps.shape[1]
                pss_lo = rope_t.tile([64, 512], BF16, tag="pss_lo")
                pss_hi = rope_t.tile([64, 512], BF16, tag="pss_hi")
                if dve_bounce:
                    nc.vector.tensor_copy(pss_lo[:, :w], ps[0:64, :])
                    nc.vector.tensor_copy(pss_hi[:, :w], ps[64:128, :])
                else:
                    nc.scalar.activation(pss_lo[:, :w], ps[0:64, :],
                                         mybir.ActivationFunctionType.Copy)
                    nc.scalar.activation(pss_hi[:, :w], ps[64:128, :],
                                         mybir.ActivationFunctionType.Copy)
                t0 = rope_t.tile([64, 512], BF16, tag="t0")
                t1 = rope_t.tile([64, 512], BF16, tag="t1")
                t2 = rope_t.tile([64, 512], BF16, tag="t2")
                t3 = rope_t.tile([64, 512], BF16, tag="t3")
                nc.vector.tensor_mul(t0[:, :w], pss_lo[:, :w], cs)
                nc.vector.tensor_mul(t1[:, :w], pss_hi[:, :w], sn)
                nc.vector.tensor_sub(dst[0:64, sl], t0[:, :w], t1[:, :w])
                nc.vector.tensor_mul(t2[:, :w], pss_lo[:, :w], sn)
                nc.vector.tensor_mul(t3[:, :w], pss_hi[:, :w], cs)
                nc.vector.tensor_add(dst[64:128, sl], t2[:, :w], t3[:, :w])

            # PSUM budget (8 banks): fill 3 + scores 3 + attn-out 2. The
            # triple-buffered scores pool lets the PE run two score tiles
            # ahead of the (loaded) scalar-engine exp queue.
            with (
                tc.tile_pool(name="fill_ps", bufs=3, space="PSUM") as fill_ps,
                tc.tile_pool(name="s_ps", bufs=3, space="PSUM") as s_ps,
                tc.tile_pool(name="o_ps", bufs=2, space="PSUM") as o_ps,
                tc.tile_pool(name="rs_sb", bufs=2) as rs_sb,
            ):
                def xh_ap(p, c0, c1):
                    # xh pair access; pair 0 is split into 1024-col piece tiles
                    if p == 0:
                        pc = c0 // 1024
                        assert c1 <= (pc + 1) * 1024
                        return xh0_p[pc][:, :, c0 - pc * 1024:c1 - pc * 1024]
                    return xh_t[p][:, :, c0:c1]

                def emit_proj_mm(ps, job, p, term, start, stop):
                    # terms 0/1 read xh only; term 2 reads xl (so the eager
                    # loop can emit 0/1 before the xl pair lands).
                    # k/q: 0 = wh@xh, 1 = wl@xh, 2 = wh@xl
                    # v:   0 = xh@wvh, 1 = xh@wvl, 2 = xl@wvh
                    kind, h, idx = job
                    if kind == "v":
                        xt = (xl_t[p][:, :, idx * 128:(idx + 1) * 128]
                              if term == 2 else
                              xh_ap(p, idx * 128, (idx + 1) * 128))
                        wt = (wvh_t, wvl_t, wvh_t)[term][p]
                        nc.tensor.matmul(ps[:, 0:HD], xt, wt[:], start=start,
                                         stop=stop, perf_mode=DR,
                                         skip_group_check=True)
                        return
                    xt = (xl_t[p][:, :, idx * 512:(idx + 1) * 512]
                          if term == 2 else
                          xh_ap(p, idx * 512, (idx + 1) * 512))
                    if kind == "k":
                        wt = (wkh_t, wkl_t, wkh_t)[term][p]
                        nc.tensor.matmul(ps[:], wt[:], xt, start=start,
                                         stop=stop, perf_mode=DR,
                                         skip_group_check=True)
                    else:
                        hsl = slice(h * 128, (h + 1) * 128)
                        wt = (wqh_t, wql_t, wqh_t)[term][p]
                        nc.tensor.matmul(ps[:], wt[:, :, hsl], xt, start=start,
                                         stop=stop, perf_mode=DR,
                                         skip_group_check=True)

                def emit_proj_store(ps, job, v_scalar=False):
                    kind, h, idx = job
                    if kind == "k":
                        rope_store(ps, kT_t, slice(idx * 512, (idx + 1) * 512))
                    elif kind == "v":
                        if v_scalar:
                            nc.scalar.activation(
                                v_t[idx][:], ps[:, 0:HD],
                                mybir.ActivationFunctionType.Copy)
                        else:
                            nc.vector.tensor_copy(v_t[idx][:], ps[:, 0:HD])
                    else:
                        rope_store(ps, qT_t[h], slice(idx * 512, (idx + 1) * 512))

                def emit_proj_job(ps, job):
                    # full 24-instruction emission (fill path)
                    n = 0
                    for p in range(NPD):
                        for term in range(3):
                            emit_proj_mm(ps, job, p, term,
                                         start=(n == 0), stop=(n == 3 * NPD - 1))
                            n += 1
                            yield 1

                # -- eager: per pair, K(4 blocks) then V(lt 0-3), 3 terms
                # each, tracking the xh/xl pair stream.
                kb = [("k", 0, nk) for nk in range(NQT)]
                kp = [(fill_ps, "f"), (fill_ps, "f"), (s_ps, "scores"),
                      (s_ps, "scores")]
                ktiles = [pool.tile([128, 512], F32, tag=t, name=f"pjk{i}")
                          for i, (pool, t) in enumerate(kp)]
                vb = [("v", 0, lt) for lt in range(4)]
                vp = [(o_ps, "aout"), (o_ps, "aout"),
                      (s_ps, "scores"), (fill_ps, "f")]
                vtiles = [pool.tile([128, 512], F32, tag=t, name=f"pjv{i}")
                          for i, (pool, t) in enumerate(vp)]
                for p in range(NPD):
                    st = p == 0
                    sp = p == NPD - 1
                    for term in (0, 1):
                        for ps, job in zip(ktiles, kb):
                            emit_proj_mm(ps, job, p, term,
                                         start=(st and term == 0), stop=False)
                    for ps, job in zip(vtiles, vb):
                        emit_proj_mm(ps, job, p, 0, start=st, stop=False)
                        emit_proj_mm(ps, job, p, 1, start=False, stop=False)
                    # lo-x terms (term 2) after xl_p arrival
                    for ps, job in zip(ktiles, kb):
                        emit_proj_mm(ps, job, p, 2, start=False, stop=sp)
                    for ps, job in zip(vtiles, vb):
                        emit_proj_mm(ps, job, p, 2, start=False, stop=sp)
                # k0/k1 stores free fill_ps slots early; scalar bounces (the
                # DVE is the eager-tail critical path with V copies + ropes).
                rope_store(ktiles[0], kT_t, slice(0, 512))
                rope_store(ktiles[1], kT_t, slice(512, 1024))
                # -- eager: Q projections for block 0 heads 0-1. The V-psum
                # copies come AFTER the q rope stores on the DVE: v tiles are
                # first read well into block 0, qT gates the first scores.
                for h in range(2):
                    ps = fill_ps.tile([128, 512], F32, tag="f")
                    for _ in emit_proj_job(ps, ("q", h, 0)):
                        pass
                    emit_proj_store(ps, ("q", h, 0))
                for ps, job in zip(vtiles, vb):
                    emit_proj_store(ps, job)
                emit_proj_store(ktiles[2], kb[2])
                emit_proj_store(ktiles[3], kb[3])

                # -- fill generators
                proj_rest = [("q", 2, 0), ("q", 3, 0)]
                for nqq in range(1, NQT):
                    proj_rest.append(("q", 0, nqq))
                    proj_rest.append(("v", 0, 4 * nqq))
                    proj_rest.append(("v", 0, 4 * nqq + 1))
                    proj_rest.append(("q", 1, nqq))
                    proj_rest.append(("v", 0, 4 * nqq + 2))
                    proj_rest.append(("v", 0, 4 * nqq + 3))
                    proj_rest.append(("q", 2, nqq))
                    proj_rest.append(("q", 3, nqq))
                proj_done = [0]   # jobs fully emitted (for force-drain)

                def proj_gen():
                    for job in proj_rest:
                        ps = fill_ps.tile([128, 512], F32, tag="f")
                        yield from emit_proj_job(ps, job)
                        emit_proj_store(ps, job)
                        proj_done[0] += 1
                        yield 1

                def wo_emit_half(ps, P, lsl, osl, start, stop):
                    n = 0
                    for lh, rh in ((aoh_t, woh_t), (aol_t, woh_t),
                                   (aoh_t, wol_t)):
                        nc.tensor.matmul(
                            ps[:], lh[P][:, :, lsl], rh[P][:, :, osl],
                            start=(start and n == 0),
                            stop=(stop and n == 2),
                            perf_mode=DR, skip_group_check=True,
                        )
                        n += 1

                orow = [None]

                def wo_finish(ps, lt, no, lsl, osl, alt_dma=False):
                    # accumulate a full [128, 2048] output row in SBUF and
                    # DMA it once (4 small transfers pay the 500ns floor;
                    # one 512KB row transfer streams at full bandwidth).
                    if no == 0:
                        orow[0] = out_sb.tile([128, L], BF16, tag="out", name="orow")
                    ot = orow[0]
                    # gpsimd can't read PSUM: alternate the bounce
                    # between the DVE and the scalar engine. The very last
                    # row puts its final quarter on the scalar engine and
                    # its row DMA on the scalar queue: the DMA's SEQ wait is
                    # then satisfied in-order (no Act-queue hold) and the
                    # last two 512KB transfers overlap across queues.
                    last_row = lt == NLT - 1
                    dve_side = (no % 2 == 0) if last_row else ((lt + no) % 2 == 0)
                    if dve_side:
                        nc.vector.tensor_copy(ot[:, osl], ps[:])
                    else:
                        nc.scalar.activation(
                            ot[:, osl], ps[:],
                            mybir.ActivationFunctionType.Copy)
                    if no == NQT - 1:
                        (nc.scalar if last_row else nc.sync).dma_start(
                            out[lsl, :], ot[:])

                def wo_gen_flat(nq_blk):
                    for lt in range(4 * nq_blk, 4 * nq_blk + 4):
                        lsl = slice(lt * 128, (lt + 1) * 128)
                        for no in range(NQT):
                            osl = slice(no * 512, (no + 1) * 512)
                            ps = fill_ps.tile([128, 512], F32, tag="f")
                            n = 0
                            for P in range(2):
                                for lh, rh in ((aoh_t, woh_t), (aol_t, woh_t),
                                               (aoh_t, wol_t)):
                                    nc.tensor.matmul(
                                        ps[:], lh[P][:, :, lsl],
                                        rh[P][:, :, osl],
                                        start=(n == 0), stop=(n == 5),
                                        perf_mode=DR, skip_group_check=True,
                                    )
                                    n += 1
                                    yield 1
                            wo_finish(ps, lt, no, lsl, osl)
                            yield 1

                def wo_gen_pipelined(nq_blk, depth=8):
                    # Tail generator for the last block. Block 3's heads run
                    # in order [2,3,0,1], so pair-1 (heads 2/3) is finalized
                    # three heads early: emit all pair-1 halves first,
                    # pipelined `depth` tiles deep across the (now free)
                    # fill/scores/aout psum pools, so the last head's
                    # finalize latency is covered with ready PE work.
                    tiles = [(lt, no) for lt in range(4 * nq_blk, 4 * nq_blk + 4)
                             for no in range(NQT)]
                    # first 3 tiles use fill_ps (safe to allocate while the
                    # last head's attention still runs); the rest borrow the
                    # scores/aout pools, which are free only in the tail.
                    # o_ps last: its buffers stay WAR-blocked until the
                    # final head's fin_mul reads its pso in the tail.
                    pools = [(fill_ps, "f"), (fill_ps, "f"), (fill_ps, "f"),
                             (s_ps, "scores"), (s_ps, "scores"),
                             (s_ps, "scores"), (o_ps, "aout"), (o_ps, "aout")]
                    pss = {}

                    def pA(t):
                        lt, no = tiles[t]
                        lsl = slice(lt * 128, (lt + 1) * 128)
                        osl = slice(no * 512, (no + 1) * 512)
                        pool, tag = pools[t % depth]
                        ps = pool.tile([128, 512], F32, tag=tag)
                        pss[t] = (ps, lsl, osl)
                        wo_emit_half(ps, 1, lsl, osl, True, False)

                    def pB(t):
                        lt, no = tiles[t]
                        ps, lsl, osl = pss[t]
                        wo_emit_half(ps, 0, lsl, osl, False, True)
                        wo_finish(ps, lt, no, lsl, osl)

                    for t in range(depth):
                        pA(t)
                        yield 1
                        yield 1
                        yield 1
                    for t in range(len(tiles)):
                        pB(t)
                        yield 1
                        yield 1
                        yield 1
                        yield 1
                        if t + depth < len(tiles):
                            pA(t + depth)
                            yield 1
                            yield 1
                            yield 1

                fill_q = [["proj", proj_gen(), 0]]

                def drain(n, wo_cap=None, wo3_cap=None):
                    # drain up to n fill micro-ops, preserving FIFO order.
                    # wo_cap limits the WITHIN-TILE op index taken from a wo
                    # generator: ops 3..7 of each 8-op tile (the pair-1
                    # matmuls reading heads 2/3, + copy/dma) must not be
                    # emitted before the previous block's last-head finalize.
                    # wo3_cap limits TOTAL ops from the early-appended block-3
                    # generator to its ready pair-1 prefix.
                    while n > 0 and fill_q:
                        ent = fill_q[0]
                        if (ent[0] == "wo" and wo_cap is not None
                                and ent[2] % 8 >= wo_cap):
                            return
                        if (ent[0] == "wo3" and wo3_cap is not None
                                and ent[2] >= wo3_cap):
                            return
                        if next(ent[1], None) is None:
                            fill_q.pop(0)
                        else:
                            ent[2] += 1
                            n -= 1

                def force_proj(njobs):
                    # ensure the first njobs of proj_rest are fully emitted
                    while proj_done[0] < njobs:
                        drain(80, wo_cap=0)
                        if not fill_q or fill_q[0][0] != "proj":
                            break

                # Deferred head finalization, staged across the NEXT head's
                # tile loop (mk 0/1/3/4) so neither the in-order PE nor the
                # scalar engine's exp queue ever waits on it: the aoh copy
                # (scalar) lands two tiles after its DVE input is produced.
                fin_pending = None  # (pso, acc, h, nq)

                def fin_stage1(pso, acc, h, nq):
                    # bf16 rowsum/reciprocal: the 2-byte dtype gets the DVE
                    # 2x mode (297 vs 594ns) and shortens the last-head
                    # finalize chain; the ~0.2% denominator rounding is far
                    # inside the error budget.
                    rs = rs_sb.tile([128, 512], F32, tag="rs")
                    nc.gpsimd.partition_all_reduce(
                        rs[:], acc[:], channels=128,
                        reduce_op=bass_isa.ReduceOp.add)
                    rc = fin_sb.tile([128, 512], F32, tag="recip")
                    nc.vector.reciprocal(rc[:], rs[:])
                    return rc

                def fin_mul(pso, rc):
                    t = fin_sb.tile([128, 512], BF16, tag="nt")
                    nc.vector.tensor_mul(t[:], pso[:], rc[:])
                    return t

                def fin_hi(t, h, nq):
                    P, i = divmod(h, 2)
                    nc.scalar.activation(aoh_t[P][:, i, qsl_of(nq)], t[:],
                                         mybir.ActivationFunctionType.Copy)

                def fin_lo(t, h, nq):
                    P, i = divmod(h, 2)
                    qs = qsl_of(nq)
                    nc.vector.tensor_sub(aol_t[P][:, i, qs], t[:],
                                         aoh_t[P][:, i, qs])

                for nq in range(NQT):
                    nmk = 4 * (nq + 1)   # causal: k tiles 0..nmk-1
                    if nq >= 1:
                        force_proj(2 + 8 * nq)

                    def col0(mk):
                        return 128 * (mk - 4 * nq) if mk >= 4 * nq else 0

                    horder = [2, 3, 0, 1] if nq == NQT - 1 else list(range(HQ))
                    for hi_, h in enumerate(horder):
                        if nq == 0 and hi_ >= 2:
                            force_proj(hi_ - 1)
                        if hi_ == 2 and nq <= 2:
                            force_proj(3 + 8 * nq)
                        if hi_ == 3 and nq <= 2:
                            force_proj(6 + 8 * nq)
                        pso = o_ps.tile([128, 512], F32, tag="aout")
                        acc = rs_sb.tile([128, 512], BF16, tag="acc")

                        def emit_scores(mk):
                            c0 = col0(mk)
                            ksl = slice(mk * 128, (mk + 1) * 128)
                            ps = s_ps.tile([128, 512], F32, tag="scores")
                            nc.tensor.matmul(
                                ps[:, c0:], kT_t[:, ksl],
                                qT_t[h][:, nq * 512 + c0:(nq + 1) * 512],
                                start=True, stop=True,
                            )
                            return ps

                        if nq == 0 and hi_ == 0:
                            # cover the eager-phase DVE rope tail (kT/qT
                            # stores) with fill matmuls before first scores
                            drain(26, wo_cap=0)
                        fin_rc = None
                        fin_t = None
                        ps_cur = emit_scores(0)
                        for mk in range(nmk):
                            c0 = col0(mk)
                            at = acc if mk == 0 else attn_sb.tile(
                                [128, 512], BF16, tag="attnT")
                            nc.scalar.activation(
                                at[:, c0:], ps_cur[:, c0:],
                                mybir.ActivationFunctionType.Exp,
                                scale=SCALE / (WS_QK * WS_QK),
                            )
                            if mk >= 4 * nq:
                                nc.gpsimd.affine_select(
                                    out=at[:, c0:], in_=at[:, c0:],
                                    compare_op=mybir.AluOpType.is_ge,
                                    fill=0.0,
                                    base=0,
                                    pattern=[[1, 512 - c0]],
                                    channel_multiplier=-1,
                                )
                            if mk + 1 < nmk:
                                ps_cur = emit_scores(mk + 1)
                            drain(5, wo_cap=(3 if (hi_ == 0 and mk < 3) else None),
                                  wo3_cap=9)
                            nc.tensor.matmul(
                                pso[:, c0:], v_t[mk][:], at[:, c0:],
                                start=(mk == 0), stop=(mk == nmk - 1),
                                skip_group_check=True,
                            )
                            if mk > 0:
                                nc.vector.tensor_add(
                                    acc[:, c0:], acc[:, c0:], at[:, c0:])
                            if fin_pending is not None:
                                if mk == 0:
                                    fin_rc = fin_stage1(*fin_pending)
                                elif mk == 1:
                                    fin_t = fin_mul(fin_pending[0], fin_rc)
                                elif mk == 2:
                                    fin_hi(fin_t, fin_pending[2], fin_pending[3])
                                elif mk == 3:
                                    fin_lo(fin_t, fin_pending[2], fin_pending[3])
                                    fin_pending = None

                        fin_pending = (pso, acc, h, nq)
                        if nq == NQT - 1 and hi_ == 2:
                            # block 3's wo: its pair-1 halves (heads 2/3,
                            # order [2,3,0,1]) are finalized by the end of
                            # the 3rd head -- let them drain during the 4th
                            # head's tile loop.
                            fill_q.append(["wo3", wo_gen_pipelined(nq), 0])

                    if nq != NQT - 1:
                        fill_q.append(["wo", wo_gen_flat(nq), 0])

                # final head finalize + leftover fill work. At most 3 wo ops
                # may be drained before the last ao block is written.
                rc_last = fin_stage1(*fin_pending)
                t_last = fin_mul(fin_pending[0], rc_last)
                drain(24, wo3_cap=24)
                fin_hi(t_last, fin_pending[2], fin_pending[3])
                fin_lo(t_last, fin_pending[2], fin_pending[3])
                fin_pending = None
                while fill_q:
                    drain(1000)

    nc.compile()
    return nc


_ROPE_PERM = np.concatenate([np.arange(0, HD, 2), np.arange(1, HD, 2)])


def _split8(x):
    h = np.asarray(x, np.float32).astype(E4)
    l = (np.asarray(x, np.float32) - h.astype(np.float32)).astype(E4)
    return h, l


def _pack_pairs(w, npairs):
    """[npairs*2*128, W] -> DoubleRow pair layout [128, npairs*2*W]."""
    W = w.shape[1]
    return np.ascontiguousarray(
        w.reshape(npairs, 2, 128, W).transpose(2, 0, 1, 3).reshape(128, -1))


def _prep_inputs(x, freqs_cos, freqs_sin, Wq, Wk, Wv, Wo):
    """Build the 8 per-core input maps (numpy, host-side)."""
    x = np.asarray(x, np.float32)
    cosT = np.ascontiguousarray(np.asarray(freqs_cos, np.float32).T).astype(BF)
    sinT = np.ascontiguousarray(np.asarray(freqs_sin, np.float32).T).astype(BF)
    Wq = np.asarray(Wq, np.float32)
    Wk = np.asarray(Wk, np.float32)
    Wv = np.asarray(Wv, np.float32)
    Wo = np.asarray(Wo, np.float32)

    xT_b = [np.ascontiguousarray(x[b].T) for b in range(B)]
    xhl_b = [_split8(t) for t in xT_b]

    in_maps = []
    for c in range(8):
        b, t = divmod(c, TP)
        wq_c = Wq[:, t * HQ * HD:(t + 1) * HQ * HD].reshape(D, HQ, HD)
        wq_c = np.ascontiguousarray(wq_c[:, :, _ROPE_PERM].reshape(D, HQ * HD)) * WS_QK
        wk_c = np.ascontiguousarray(Wk[:, t * HD:(t + 1) * HD][:, _ROPE_PERM]) * WS_QK
        wv_c = np.ascontiguousarray(Wv[:, t * HD:(t + 1) * HD]) * WS_V
        wo_c = np.ascontiguousarray(Wo[t * HQ * HD:(t + 1) * HQ * HD, :]) * WS_O
        wqh, wql = _split8(wq_c)
        wkh, wkl = _split8(wk_c)
        wvh, wvl = _split8(wv_c)
        woh, wol = _split8(wo_c)
        xh, xl = xhl_b[b]
        in_maps.append({
            "xh": xh, "xl": xl,
            "wqh": _pack_pairs(wqh, NPD), "wql": _pack_pairs(wql, NPD),
            "wkh": _pack_pairs(wkh, NPD), "wkl": _pack_pairs(wkl, NPD),
            "wvh": _pack_pairs(wvh, NPD), "wvl": _pack_pairs(wvl, NPD),
            "woh": _pack_pairs(woh, 2), "wol": _pack_pairs(wol, 2),
            "cosT": cosT,
            "sinT": sinT,
        })
    return in_maps


_NC_CACHE = None


def run(inputs, trace=False, trace_kwargs=None):
    global _NC_CACHE
    if _NC_CACHE is None:
        _NC_CACHE = build_nc()
    nc = _NC_CACHE
    in_maps = _prep_inputs(
        inputs["x"], inputs["freqs_cos"], inputs["freqs_sin"],
        inputs["Wq"], inputs["Wk"], inputs["Wv"], inputs["Wo"],
    )
    try:
        res = bass_utils.run_bass_kernel_spmd(
            nc, in_maps, core_ids=list(range(8)),
            trace=trace, **(trace_kwargs or {}),
        )
    except ModuleNotFoundError:
        res = bass_utils.run_bass_kernel_spmd(
            nc, in_maps, core_ids=list(range(8)), trace=False,
        )
    partials = [r["out"] for r in res.results]
    out = np.empty((B, L, D), np.float32)
    inv = 1.0 / (WS_V * WS_O)   # undo the host-side weight scaling (ao*Wo)
    for b in range(B):
        acc = partials[b * TP].astype(np.float32)
        for t in range(1, TP):
            acc = acc + partials[b * TP + t]
        out[b] = acc * inv
    # exact host-side bias folds: +bo, and +bv @ Wo (softmax rows sum to 1,
    # so v-bias contributes attn@1 * bv = bv per row, through Wo).
    bo = np.asarray(inputs["bo"], np.float32)
    bv = np.asarray(inputs["bv"], np.float32)
    Wo = np.asarray(inputs["Wo"], np.float32)
    bias = bo + np.repeat(bv.reshape(KVH, HD), N_REP, axis=0).reshape(-1) @ Wo
    out += bias[None, None, :]
    return out, res


def kernel(**inputs) -> np.ndarray:
    out, _ = run(inputs, trace=False)
    return out


if __name__ == "__main__":
    pass
